# revision 45
# baseline (speedup 1.0000x reference)
"""Trainium2 Bass kernel for nn_ContinuousConvolutionBlock (gnn_message_passing).

Strategy (per sharding hint: partition points across 8 cores; each core owns its
queries' scatter-reduce and tap-GEMM; filter + dense weights replicated):

Host side (index plumbing / input marshalling only — zero FLOPs):
  - qry_idx is sorted; queries are grouped into 8-query blocks, blocks paired
    into 128-edge-slot "chunks" (two-pointer bin packing, ~3% padding).
  - Consecutive block ranges are assigned to the 8 cores; per-core per-slot
    payload arrays (pos[src], pos[qry], feats[src] (bf16), local query id) are
    marshalled on host and DMA'd in dense [128 x NCH x k] layout.

Device side (all FLOP-bearing compute):
  - Geometry: ball->cube volume-preserving map (DVE arithmetic + ACT
    sqrt/arctan/sign/abs) on UNSCALED relative coords (map is linear in scale,
    folded into the grid transform), then trilinear corner weights via the
    hat function w[ax] = relu(1 - |g - ax|) evaluated with bf16 2x-packed
    DVE tensor_scalar ops, duplicated x2 along the tap axis ("dup-pair"
    packing) so downstream DVE ops hit the 2x packed mode.
  - The geometry dependency chain is latency-bound, so the chunk range is cut
    into segments whose chains are emitted interleaved (window of 2) to keep
    DVE/ACT throughput-bound.
  - Scatter-reduce as factored matmul per 128-slot chunk: with
    R[slot,(ax,c)] = w4x (x) feats  (bf16) and
    L[slot,(t,hq)]  = zy (x) Qoh    (bf16, t=(az,ay), hq = query-in-chunk),
    PE computes A^T[(ax,c),(t,hq)] = R^T @ L per chunk into PSUM.
  - PSUM->SBUF cast-copies re-arrange A into at[(ax,c), (t, chunk, hq)] bf16
    so each tap-GEMM rhs slice is fully contiguous.
  - Tap-GEMM: out^T += G_t^T @ at[:, t-slice], accumulated over 16 t in PSUM,
    batched over pairs of 16-chunk groups (512-col rhs).
  - Dense branch: out_dense^T = dense_w^T @ feats^T (+bias via ACT) on PE.
  Outputs are produced transposed ([64, nq]); host transposes/reorders back.
"""
import sys
import os
sys.path.insert(0, '/opt/trn_rl_repo')
import numpy as np
import ml_dtypes

N = 30000
CIN = 32
COUT = 64
KS = 4
EXTENT = 0.08
NCORES = 8
NBLK = N // 8  # 3750 eight-query blocks

BF16 = ml_dtypes.bfloat16

_COMPILED = {}

# tuning knobs
SEG_GROUPS = [2, 6, 7]         # geometry segment sizes (in 16-chunk groups)
WINDOW = 2                     # interleaved segment chains
COPY_DVE = lambda g, q: False  # quad copies all on ACT (DVE is the binding engine)
TAP_SPAN = 2                   # groups per tap-GEMM batch (512-col rhs)
# arctan(t) ~ t*(C0 + C1 u + C2 u^2 + C3 u^3 + C4 u^4), u = t^2, |t| <= 1
ATAN_C = (0.9999593504130755, -0.3313379208450923, 0.18331602986643833,
          -0.0887740260254787, 0.022239144264532117)


# ----------------------------------------------------------------------------
# Host planning
# ----------------------------------------------------------------------------
def _plan(qry_idx):
    deg = np.bincount(qry_idx, minlength=N)
    bsz = deg.reshape(NBLK, 8).sum(1)
    bstart = np.concatenate([[0], np.cumsum(bsz)]).astype(np.int64)
    per = [NBLK // NCORES + (1 if c < NBLK % NCORES else 0) for c in range(NCORES)]
    b0 = np.concatenate([[0], np.cumsum(per)]).astype(np.int64)
    plans = []
    for c in range(NCORES):
        blocks = list(range(b0[c], b0[c + 1]))
        asc = sorted(blocks, key=lambda b: bsz[b])
        chunks = []
        lo, hi = 0, len(asc) - 1
        while lo <= hi:
            if lo == hi:
                chunks.append((asc[hi], None)); break
            if bsz[asc[hi]] + bsz[asc[lo]] <= 128:
                chunks.append((asc[hi], asc[lo])); hi -= 1; lo += 1
            else:
                chunks.append((asc[hi], None)); hi -= 1
        plans.append(dict(blocks=blocks, chunks=chunks, q0=int(8 * b0[c]),
                          nq=int(8 * (b0[c + 1] - b0[c]))))
    return plans, bstart, bsz


def _pack_core(plan_c, bstart, pos, feats, qry_idx, src_idx, NCHP):
    """Build per-slot payload arrays in [128, NCHP, k] layout."""
    possrc = np.zeros((128, NCHP, 3), np.float32)
    posqry = np.zeros((128, NCHP, 3), np.float32)
    fsrc = np.zeros((128, NCHP, CIN), BF16)
    qlocf = np.full((128, NCHP, 2), -1.0, BF16)
    for ci, (bA, bB) in enumerate(plan_c['chunks']):
        s = 0
        for half, b in enumerate((bA, bB)):
            if b is None:
                continue
            e0, e1 = int(bstart[b]), int(bstart[b + 1])
            n = e1 - e0
            sl = slice(s, s + n)
            possrc[sl, ci, :] = pos[src_idx[e0:e1]]
            posqry[sl, ci, :] = pos[qry_idx[e0:e1]]
            fsrc[sl, ci, :] = feats[src_idx[e0:e1]].astype(BF16)
            ql = ((qry_idx[e0:e1] - 8 * b) + 8 * half).astype(BF16)
            qlocf[sl, ci, 0] = ql
            qlocf[sl, ci, 1] = ql
            s += n
    return possrc, posqry, fsrc, qlocf


# ----------------------------------------------------------------------------
# Device kernel
# ----------------------------------------------------------------------------
def _build_bass(NCHP, NQ):
    import concourse.bass as bass
    import concourse.tile as tile
    from concourse import bacc, mybir
    from concourse.bass import AP

    f32 = mybir.dt.float32
    bf16 = mybir.dt.bfloat16
    f16 = mybir.dt.float16
    i32 = mybir.dt.int32
    ALU = mybir.AluOpType
    ACT = mybir.ActivationFunctionType
    EPS = 1e-12
    F4PI = float(4.0 / np.pi)
    GSCL = float(1.5 * 2.0 / EXTENT)  # grid scale folded: g = GSCL*m + 1.5

    nc = bacc.Bacc("TRN2", target_bir_lowering=False, debug=False)

    possrc = nc.dram_tensor("possrc", (128, NCHP, 3), f32, kind="ExternalInput")
    posqry = nc.dram_tensor("posqry", (128, NCHP, 3), f32, kind="ExternalInput")
    fsrc = nc.dram_tensor("fsrc", (128, NCHP, CIN), bf16, kind="ExternalInput")
    qlocf = nc.dram_tensor("qlocf", (128, NCHP, 2), bf16, kind="ExternalInput")
    g2 = nc.dram_tensor("g2", (128, 16 * 64), bf16, kind="ExternalInput")
    featsT = nc.dram_tensor("featsT", (CIN, NQ), bf16, kind="ExternalInput")
    denw = nc.dram_tensor("denw", (CIN, COUT), bf16, kind="ExternalInput")
    denb = nc.dram_tensor("denb", (COUT, 1), f32, kind="ExternalInput")

    outconvT = nc.dram_tensor("outconvT", (COUT, NQ), f32, kind="ExternalOutput")
    outdenseT = nc.dram_tensor("outdenseT", (COUT, NQ), f32, kind="ExternalOutput")

    W = NCHP
    NGRP = NCHP // 16
    assert sum(SEG_GROUPS) == NGRP, (SEG_GROUPS, NGRP)
    segs = []
    g0 = 0
    for n in SEG_GROUPS:
        segs.append((g0, g0 + n))
        g0 += n

    with tile.TileContext(nc) as tc:
        with tc.tile_pool(name="inp", bufs=1) as inp, \
             tc.tile_pool(name="geo", bufs=1) as geo, \
             tc.tile_pool(name="tmp", bufs=1) as tmp, \
             tc.tile_pool(name="lp", bufs=3) as lpool, \
             tc.tile_pool(name="rp", bufs=4) as rpool, \
             tc.tile_pool(name="at", bufs=3) as atp, \
             tc.tile_pool(name="outp", bufs=2) as outp, \
             tc.tile_pool(name="ps1", bufs=3, space="PSUM") as ps1, \
             tc.tile_pool(name="ps2", bufs=2, space="PSUM") as ps2:

            # ---------------- input DMAs (priority order) ----------------
            t_ps = inp.tile([128, W, 3], f32)
            t_pq = inp.tile([128, W, 3], f32)
            t_f = inp.tile([128, W, CIN], bf16)
            t_ql = inp.tile([128, W, 2], bf16)
            t_g2 = inp.tile([128, 16 * 64], bf16)
            t_ftT = inp.tile([CIN, NQ], bf16)
            t_dw = inp.tile([CIN, COUT], bf16)
            t_db = inp.tile([COUT, 1], f32)
            nc.sync.dma_start(t_ps[:], possrc[:])
            nc.sync.dma_start(t_pq[:], posqry[:])
            nc.sync.dma_start(t_ql[:], qlocf[:])
            nc.sync.dma_start(t_ftT[:], featsT[:])
            nc.sync.dma_start(t_dw[:], denw[:])
            nc.sync.dma_start(t_db[:], denb[:])
            nc.sync.dma_start(t_f[:], fsrc[:])
            nc.sync.dma_start(t_g2[:], g2[:])

            # iota constants
            io16i = tmp.tile([128, 16], i32)
            nc.gpsimd.iota(io16i[:], pattern=[[1, 16]], base=0, channel_multiplier=0)
            io16b = geo.tile([128, 16], bf16)
            nc.vector.tensor_copy(io16b[:], io16i[:])
            # c4m = [0,0,1,1,2,2,3,3] - 1.5  (dup-pair tap offsets)
            c4di = tmp.tile([128, 8], i32)
            nc.gpsimd.iota(c4di[:], pattern=[[1, 4], [0, 2]], base=0,
                           channel_multiplier=0)
            c4m = geo.tile([128, 8], f32)
            nc.vector.tensor_copy(c4m[:], c4di[:])
            nc.vector.tensor_scalar(c4m[:], c4m[:], 1.5, None, op0=ALU.subtract)
            ceps = geo.tile([128, 1], f32)
            nc.gpsimd.memset(ceps[:], EPS)

            # ---------------- dense branch (bf16 matmul, runs first) --------
            NSEG = (NQ + 511) // 512
            for s in range(NSEG):
                j0 = s * 512
                j1 = min(NQ, j0 + 512)
                pd = ps2.tile([COUT, 512], f32, space="PSUM", tag="po")
                nc.tensor.matmul(
                    out=pd[:, 0:j1 - j0],
                    lhsT=t_dw[:],
                    rhs=t_ftT[:, j0:j1],
                    start=True, stop=True)
                odt = outp.tile([COUT, 512], f32, tag="odst")
                db = t_db[:, 0:1]
                nc.scalar.activation(odt[:, 0:j1 - j0], pd[:, 0:j1 - j0],
                                     ACT.Identity, bias=db, scale=1.0)
                nc.sync.dma_start(outdenseT[:, j0:j1], odt[:, 0:j1 - j0])

            # ---------------- temp tile machinery (per-namespace) ----------
            _tn = [0]
            _free_tags = {}
            _tag_of = {}
            _seq = [0]

            def T(ns, shape, dt_=f32):
                key = tuple(shape) + (dt_,)
                free = _free_tags.setdefault(ns, [])
                for i, (tg, k) in enumerate(free):
                    if k == key:
                        free.pop(i)
                        break
                else:
                    _tn[0] += 1
                    tg = f"{ns}t{_tn[0]}"
                _seq[0] += 1
                t = tmp.tile(list(shape), dt_, name=f"{tg}_u{_seq[0]}", tag=tg)
                _tag_of[id(t)] = (ns, tg, key)
                return t

            def F(*ts):
                for t in ts:
                    ns, tg, key = _tag_of.pop(id(t))
                    _free_tags[ns].append((tg, key))

            TT = nc.vector.tensor_tensor
            TS = nc.vector.tensor_scalar
            STT = nc.vector.scalar_tensor_tensor

            # delayed tap-GEMM state for PE software pipelining
            pend = []

            def flush_tap():
                if not pend:
                    return
                at_t, g0p, np_ = pend.pop(0)
                cols = np_ * 256
                po = ps2.tile([COUT, 512], f32, space="PSUM", tag="po")
                for t in range(16):
                    nc.tensor.matmul(
                        out=po[:, 0:cols],
                        lhsT=t_g2[:, t * 64:(t + 1) * 64],
                        rhs=at_t[:, t * 512:t * 512 + cols],
                        start=(t == 0), stop=(t == 15))
                ost = outp.tile([COUT, 512], f32, tag="ocst")
                nc.scalar.copy(ost[:, 0:cols], po[:, 0:cols])
                nc.sync.dma_start(outconvT[:, g0p * 256:g0p * 256 + cols],
                                  ost[:, 0:cols])

            def chain(ns, g_lo, g_hi):
                c0 = g_lo * 16
                Wh = (g_hi - g_lo) * 16

                # ---------------- geometry on [128, Wh] ----------------
                # fp16 I/O with fp32 islands (early squares, reciprocals).
                rs = T(ns, (128, Wh, 3))
                TT(out=rs[:], in0=t_ps[:, c0:c0 + Wh, :],
                   in1=t_pq[:, c0:c0 + Wh, :], op=ALU.subtract)
                yield
                rs6 = T(ns, (128, Wh, 3), f16)
                nc.vector.tensor_copy(rs6[:], rs[:])
                yield
                z6 = rs6[:, :, 2]

                sq3 = T(ns, (128, Wh, 3))
                TT(out=sq3[:], in0=rs[:], in1=rs[:], op=ALU.mult)
                yield
                x2, y2, z2 = sq3[:, :, 0], sq3[:, :, 1], sq3[:, :, 2]
                xy2 = T(ns, (128, Wh))
                TT(out=xy2[:], in0=x2, in1=y2, op=ALU.add)
                yield

                sq = T(ns, (128, Wh))
                TT(out=sq[:], in0=xy2[:], in1=z2, op=ALU.add)
                yield
                norm = T(ns, (128, Wh), f16)
                nc.scalar.activation(norm[:], sq[:], ACT.Sqrt)
                yield
                F(sq)
                nxy = T(ns, (128, Wh), f16)
                nc.scalar.activation(nxy[:], xy2[:], ACT.Sqrt)
                yield

                pole = T(ns, (128, Wh), f16)
                STT(out=pole[:], in0=z2, scalar=1.25, in1=xy2[:],
                    op0=ALU.mult, op1=ALU.is_gt)
                yield
                F(rs, sq3, xy2)

                azn = T(ns, (128, Wh), f16)
                nc.scalar.activation(azn[:], z6, ACT.Abs)
                yield
                den1 = T(ns, (128, Wh))
                STT(out=den1[:], in0=azn[:], scalar=EPS, in1=norm[:],
                    op0=ALU.add, op1=ALU.add)
                yield
                rd1 = T(ns, (128, Wh))
                nc.vector.reciprocal_approx_fast(rd1[:], den1[:])
                yield
                t1s = T(ns, (128, Wh), f16)
                STT(out=t1s[:], in0=norm[:], scalar=3.0, in1=rd1[:],
                    op0=ALU.mult, op1=ALU.mult)
                yield
                s1 = T(ns, (128, Wh), f16)
                nc.scalar.activation(s1[:], t1s[:], ACT.Sqrt)
                yield
                F(azn, den1, rd1, t1s)

                den2 = T(ns, (128, Wh))
                nc.scalar.activation(den2[:], nxy[:], ACT.Identity,
                                     bias=ceps[:, 0:1], scale=1.0)
                yield
                rd2 = T(ns, (128, Wh))
                nc.vector.reciprocal_approx_fast(rd2[:], den2[:])
                yield
                rd2c = T(ns, (128, Wh), f16)
                TS(rd2c[:], rd2[:], 60000.0, None, op0=ALU.min)
                yield
                s2 = T(ns, (128, Wh), f16)
                TT(out=s2[:], in0=norm[:], in1=rd2c[:], op=ALU.mult)
                yield
                F(nxy, den2, rd2, rd2c)

                d12 = T(ns, (128, Wh), f16)
                TT(out=d12[:], in0=s1[:], in1=s2[:], op=ALU.subtract)
                yield
                pw = T(ns, (128, Wh), f16)
                TT(out=pw[:], in0=pole[:], in1=d12[:], op=ALU.mult)
                yield
                wq = T(ns, (128, Wh), f16)
                TT(out=wq[:], in0=s2[:], in1=pw[:], op=ALU.add)
                yield
                F(s1, s2, d12, pw)

                xcyc = T(ns, (128, Wh, 2), f16)
                TT(out=xcyc[:], in0=rs6[:, :, 0:2],
                   in1=AP(wq.tensor, wq[:].offset, [wq[:].ap[0], [1, Wh], [0, 2]]),
                   op=ALU.mult)
                yield

                sgz = T(ns, (128, Wh), f16)
                nc.scalar.activation(sgz[:], z6, ACT.Sign)
                yield
                zcp = T(ns, (128, Wh), f16)
                TT(out=zcp[:], in0=sgz[:], in1=norm[:], op=ALU.mult)
                yield
                zce = T(ns, (128, Wh), f16)
                TS(zce[:], z6, 1.5, None, op0=ALU.mult)
                yield
                dz = T(ns, (128, Wh), f16)
                TT(out=dz[:], in0=zcp[:], in1=zce[:], op=ALU.subtract)
                yield
                pz = T(ns, (128, Wh), f16)
                TT(out=pz[:], in0=pole[:], in1=dz[:], op=ALU.mult)
                yield
                zc = T(ns, (128, Wh), f16)
                TT(out=zc[:], in0=zce[:], in1=pz[:], op=ALU.add)
                yield
                F(sgz, zcp, zce, dz, pz, pole, norm, rs6, wq)

                c2 = T(ns, (128, Wh, 2))
                TT(out=c2[:], in0=xcyc[:], in1=xcyc[:], op=ALU.mult)
                yield
                sqxy = T(ns, (128, Wh))
                TT(out=sqxy[:], in0=c2[:, :, 0], in1=c2[:, :, 1], op=ALU.add)
                yield
                nrm = T(ns, (128, Wh), f16)
                nc.scalar.activation(nrm[:], sqxy[:], ACT.Sqrt)
                yield
                F(c2, sqxy)

                axy = T(ns, (128, Wh, 2), f16)
                nc.scalar.activation(axy[:], xcyc[:], ACT.Abs)
                yield
                abr = T(ns, (128, Wh), f16)
                TT(out=abr[:], in0=axy[:, :, 1], in1=axy[:, :, 0], op=ALU.is_le)
                yield

                mm2 = T(ns, (128, Wh, 2), f16)
                TS(mm2[:], axy[:], EPS, None, op0=ALU.is_lt)
                yield
                sf2 = T(ns, (128, Wh, 2))
                TT(out=sf2[:], in0=xcyc[:], in1=mm2[:], op=ALU.add)
                yield
                F(axy, mm2)
                rcp2 = T(ns, (128, Wh, 2))
                nc.vector.reciprocal_approx_fast(rcp2[:], sf2[:])
                yield
                rat2 = T(ns, (128, Wh, 2), f16)
                # clamped cross-ratio (selected branch always has |t| <= 1)
                STT(out=rat2[:], in0=xcyc[:], scalar=1.0,
                    in1=AP(rcp2.tensor, rcp2[:].offset + 1,
                           [rcp2[:].ap[0], [2, Wh], [-1, 2]]),
                    op0=ALU.bypass, op1=ALU.mult)
                yield
                TS(rat2[:], rat2[:], 1.0, -1.0, op0=ALU.min, op1=ALU.max)
                yield
                # arctan via deg-9 odd polynomial (fp16 2x)
                uu = T(ns, (128, Wh, 2), f16)
                TT(out=uu[:], in0=rat2[:], in1=rat2[:], op=ALU.mult)
                yield
                vv = T(ns, (128, Wh, 2), f16)
                TT(out=vv[:], in0=uu[:], in1=uu[:], op=ALU.mult)
                yield
                pa = T(ns, (128, Wh, 2), f16)
                TS(pa[:], uu[:], ATAN_C[1], ATAN_C[0], op0=ALU.mult, op1=ALU.add)
                yield
                pb = T(ns, (128, Wh, 2), f16)
                TS(pb[:], uu[:], ATAN_C[3], ATAN_C[2], op0=ALU.mult, op1=ALU.add)
                yield
                STT(out=pb[:], in0=vv[:], scalar=ATAN_C[4], in1=pb[:],
                    op0=ALU.mult, op1=ALU.add)
                yield
                TT(out=pb[:], in0=vv[:], in1=pb[:], op=ALU.mult)
                yield
                TT(out=pa[:], in0=pa[:], in1=pb[:], op=ALU.add)
                yield
                at12 = T(ns, (128, Wh, 2), f16)
                TT(out=at12[:], in0=pa[:], in1=rat2[:], op=ALU.mult)
                yield
                F(uu, vv, pa, pb)
                sg2 = T(ns, (128, Wh, 2), f16)
                nc.scalar.activation(sg2[:], xcyc[:], ACT.Sign)
                yield
                F(sf2, rcp2, rat2)

                # Q = [tmpa, tmpb, xoe, yoe]
                Q = T(ns, (128, Wh, 4), f16)
                TT(out=Q[:, :, 0:2], in0=sg2[:],
                   in1=AP(nrm.tensor, nrm[:].offset,
                          [nrm[:].ap[0], [1, Wh], [0, 2]]),
                   op=ALU.mult)
                yield
                STT(out=Q[:, :, 2:4], in0=at12[:], scalar=F4PI,
                    in1=AP(Q.tensor, Q[:].offset + 1,
                           [Q[:].ap[0], [4, Wh], [-1, 2]]),
                    op0=ALU.mult, op1=ALU.mult)
                yield
                F(sg2, nrm, at12, xcyc)

                # xo = xoe + abr*(tmpa-xoe); yo = tmpb + abr*(yoe-tmpb)
                a2 = AP(Q.tensor, Q[:].offset, [Q[:].ap[0], [4, Wh], [3, 2]])
                b2 = AP(Q.tensor, Q[:].offset + 2, [Q[:].ap[0], [4, Wh], [-1, 2]])
                d2 = T(ns, (128, Wh, 2), f16)
                TT(out=d2[:], in0=a2, in1=b2, op=ALU.subtract)
                yield
                md = T(ns, (128, Wh, 2), f16)
                TT(out=md[:], in0=d2[:],
                   in1=AP(abr.tensor, abr[:].offset,
                          [abr[:].ap[0], [1, Wh], [0, 2]]),
                   op=ALU.mult)
                yield
                xoyo = T(ns, (128, Wh, 2), f16)
                TT(out=xoyo[:], in0=b2, in1=md[:], op=ALU.add)
                yield
                F(d2, md, abr)

                # ------------ hat weights, dup-pair packed bf16 ------------
                def hat_w4(m_ap, w4_t):
                    d = T(ns, (128, Wh, 8), f16)
                    STT(out=d[:],
                        in0=m_ap,
                        scalar=GSCL,
                        in1=AP(c4m.tensor, c4m[:].offset,
                               [c4m[:].ap[0], [0, Wh], [1, 8]]),
                        op0=ALU.mult, op1=ALU.subtract)
                    a = T(ns, (128, Wh, 8), f16)
                    nc.scalar.activation(a[:], d[:], ACT.Abs)
                    nc.scalar.activation(w4_t[:], a[:], ACT.Relu,
                                         bias=1.0, scale=-1.0)
                    F(d, a)

                w4x2 = geo.tile([128, Wh, 8], bf16, tag=f"w4x2_{g_lo}")
                w4y2 = T(ns, (128, Wh, 8), bf16)
                w4z2 = T(ns, (128, Wh, 8), bf16)
                hat_w4(AP(xoyo.tensor, xoyo[:].offset,
                          [xoyo[:].ap[0], [2, Wh], [0, 8]]), w4x2)
                yield
                hat_w4(AP(xoyo.tensor, xoyo[:].offset + 1,
                          [xoyo[:].ap[0], [2, Wh], [0, 8]]), w4y2)
                yield
                hat_w4(AP(zc.tensor, zc[:].offset,
                          [zc[:].ap[0], [1, Wh], [0, 8]]), w4z2)
                yield
                F(xoyo, zc)

                # zy2[slot, az*8 + ay*2 + r] = w4z[az]*w4y[ay]  (bf16 2x)
                zy2 = geo.tile([128, Wh, 32], bf16, tag=f"zy2_{g_lo}")
                for az in range(4):
                    zslc = w4z2[:, :, 2 * az:2 * az + 2]
                    TT(out=AP(zy2.tensor, zy2[:].offset + az * 8,
                              [zy2[:].ap[0], [32, Wh], [1, 8]]),
                       in0=AP(w4z2.tensor, zslc.offset,
                              [zslc.ap[0], [8, Wh], [0, 4], [1, 2]]),
                       in1=AP(w4y2.tensor, w4y2[:].offset,
                              [w4y2[:].ap[0], [8, Wh], [1, 8]]),
                       op=ALU.mult)
                    yield
                F(w4y2, w4z2)

                # qoh[slot, hq] = (qloc == hq)  (bf16 2x via dup'd qloc)
                qoh = geo.tile([128, Wh, 16], bf16, tag=f"qoh_{g_lo}")
                qslc = t_ql[:, c0:c0 + Wh, :]
                TT(out=qoh[:],
                   in0=AP(t_ql.tensor, qslc.offset,
                          [qslc.ap[0], [2, Wh], [0, 8], [1, 2]]),
                   in1=AP(io16b.tensor, io16b[:].offset,
                          [io16b[:].ap[0], [0, Wh], [1, 16]]),
                   op=ALU.is_equal)
                yield

                # ---------------- per-group builds + matmuls ----------------
                cur_at = [None]
                for g in range(g_lo, g_hi):
                    gl = g - g_lo
                    L = lpool.tile([128, 4096], bf16, tag="L")
                    TT(out=AP(L.tensor, L[:].offset,
                              [L[:].ap[0], [16, 256], [1, 16]]),
                       in0=AP(zy2.tensor, zy2[:].offset + gl * 16 * 32,
                              [zy2[:].ap[0], [2, 256], [0, 8], [1, 2]]),
                       in1=AP(qoh.tensor, qoh[:].offset + gl * 16 * 16,
                              [qoh[:].ap[0], [16, 16], [0, 16], [1, 16]]),
                       op=ALU.mult)
                    yield "b"
                    R = rpool.tile([128, 2048], bf16, tag="R")
                    TT(out=AP(R.tensor, R[:].offset,
                              [R[:].ap[0], [32, 64], [1, 32]]),
                       in0=AP(w4x2.tensor, w4x2[:].offset + gl * 16 * 8,
                              [w4x2[:].ap[0], [2, 64], [0, 16], [1, 2]]),
                       in1=AP(t_f.tensor, t_f[:].offset + g * 16 * CIN,
                              [t_f[:].ap[0], [32, 16], [0, 4], [1, 32]]),
                       op=ALU.mult)
                    yield "b"

                    p = (g - g_lo) % TAP_SPAN
                    if p == 0:
                        at_t = atp.tile([128, 16 * 512], bf16, tag="at")
                        cur_at[0] = at_t
                    else:
                        at_t = cur_at[0]
                    for q in range(4):
                        ps_t = ps1.tile([128, 1024], f32, space="PSUM", tag="s1")
                        for k in range(4):
                            ci = q * 4 + k
                            nc.tensor.matmul(
                                out=ps_t[:, k * 256:(k + 1) * 256],
                                lhsT=R[:, ci * 128:(ci + 1) * 128],
                                rhs=L[:, ci * 256:(ci + 1) * 256],
                                start=True, stop=True)
                        dst = AP(at_t.tensor,
                                 at_t[:].offset + p * 256 + q * 4 * 16,
                                 [at_t[:].ap[0], [16, 4], [512, 16], [1, 16]])
                        src = AP(ps_t.tensor, ps_t[:].offset,
                                 [ps_t[:].ap[0], [256, 4], [16, 16], [1, 16]])
                        if COPY_DVE(g, q):
                            nc.vector.tensor_copy(dst, src)
                        else:
                            nc.scalar.copy(dst, src)
                        yield "b"
                    if p == TAP_SPAN - 1 or g == g_hi - 1:
                        pend.append((at_t, g - p, p + 1))
                        flush_tap()
                    yield "b"

            # ---------------- interleaved chain scheduler ----------------
            # build-phase chains ("b" yields) advance 2 ops/round: they feed PE
            gens = [chain(f"n{i % WINDOW}", lo, hi)
                    for i, (lo, hi) in enumerate(segs)]
            active = []
            phase = {}
            nxt = 0
            while active or nxt < len(gens):
                while len(active) < WINDOW and nxt < len(gens):
                    active.append(gens[nxt])
                    nxt += 1
                for c in list(active):
                    steps = 2 if phase.get(id(c)) == "b" else 1
                    for _ in range(steps):
                        try:
                            phase[id(c)] = next(c)
                        except StopIteration:
                            active.remove(c)
                            break

            while pend:
                flush_tap()

    nc.compile()
    return nc


# ----------------------------------------------------------------------------
# Host-side input prep (shared by kernel() and test.py's profile path)
# ----------------------------------------------------------------------------
def _prepare(feats, pos, filt, dense_w, dense_b, src_idx, qry_idx):
    feats = np.ascontiguousarray(np.asarray(feats, np.float32))
    pos = np.ascontiguousarray(np.asarray(pos, np.float32))
    filt = np.asarray(filt, np.float32)
    dense_w = np.asarray(dense_w, np.float32)
    dense_b = np.asarray(dense_b, np.float32)
    src_idx = np.asarray(src_idx).astype(np.int64)
    qry_idx = np.asarray(qry_idx).astype(np.int64)

    plans, bstart, bsz = _plan(qry_idx)
    NCH = max(len(p['chunks']) for p in plans)
    NCHP = ((NCH + 15) // 16) * 16
    NQ = NCHP * 16

    # filter regroup: G2[ax*32+c, t*64+o] = filt[az, ay, ax, c, o], t = az*4+ay
    G2 = np.zeros((128, 16 * 64), np.float32)
    for az in range(4):
        for ay in range(4):
            t = az * 4 + ay
            for ax in range(4):
                G2[ax * 32:(ax + 1) * 32, t * 64:(t + 1) * 64] = filt[az, ay, ax]
    G2 = G2.astype(BF16)

    in_maps = []
    for c, p in enumerate(plans):
        possrc, posqry, fsrc, qlocf = _pack_core(p, bstart, pos, feats,
                                                 qry_idx, src_idx, NCHP)
        ftT = np.zeros((CIN, NQ), BF16)
        ftT[:, 0:p['nq']] = feats[p['q0']:p['q0'] + p['nq']].T.astype(BF16)
        in_maps.append({
            "possrc": possrc, "posqry": posqry, "fsrc": fsrc, "qlocf": qlocf,
            "g2": G2, "featsT": ftT, "denw": dense_w.astype(BF16),
            "denb": dense_b.reshape(COUT, 1).astype(np.float32),
        })
    return plans, in_maps, NCHP, NQ


# ----------------------------------------------------------------------------
# Entry point
# ----------------------------------------------------------------------------
def kernel(feats, pos, filt, dense_w, dense_b, src_idx, qry_idx):
    from concourse.bass_utils import run_bass_kernel_spmd

    plans, in_maps, NCHP, NQ = _prepare(feats, pos, filt, dense_w, dense_b,
                                        src_idx, qry_idx)

    key = (NCHP, NQ)
    if key not in _COMPILED:
        _COMPILED[key] = _build_bass(NCHP, NQ)
    nc = _COMPILED[key]

    res = run_bass_kernel_spmd(nc, in_maps, core_ids=list(range(NCORES)))

    ans_conv = np.zeros((N, COUT), np.float32)
    ans_dense = np.zeros((N, COUT), np.float32)
    for c, p in enumerate(plans):
        outT = res.results[c]["outconvT"]
        for ci, (bA, bB) in enumerate(p['chunks']):
            for half, b in enumerate((bA, bB)):
                if b is None:
                    continue
                cols = ci * 16 + half * 8
                ans_conv[8 * b:8 * b + 8] = outT[:, cols:cols + 8].T
        dT = res.results[c]["outdenseT"]
        ans_dense[p['q0']:p['q0'] + p['nq']] = dT[:, 0:p['nq']].T
    return ans_conv, ans_dense
